# revision 10
# baseline (speedup 1.0000x reference)
"""DeepSpeedMLP (residual-add -> LayerNorm -> fc1 -> ReLU -> fc2 -> residual-add)
on 8 Trainium2 NeuronCores.

Strategy (tensor-parallel, as DeepSpeed does):
  - inter_w sharded column-wise [H, I/8], output_w row-wise [I/8, H] per core.
  - Every core computes LN for all T=4096 tokens, fc1/fc2 on its I-shard,
    producing a partial fc2 output. Cross-core reduction via per-block
    AllToAll (2x the bus rate of ReduceScatter: copy descriptors instead of
    2-source reduce descriptors) + an on-chip slab-sum fused with the
    residual add, overlapped with the next block's compute.
  - Token dim processed in 5 blocks [1024,1024,1024,512,512]: bigger blocks
    halve weight re-streaming (each block touches all 32MB of weight shard);
    the small tail blocks keep the last AllToAll (the only un-hidden part)
    small.
  - Software-pipelined emission: block b+1's activation loads / residual
    adds / squares run on DVE during block b's fc2, and block b+1's LN-stat
    matmuls are interleaved into block b's fc2 instruction stream so the PE
    crosses block boundaries without idling. Engine assignment keeps each
    FIFO un-blocked: SP ring = weights + partial writebacks, Act ring =
    activation loads + PSUM evictions + epilogue loads, DVE = adds/squares/
    rstd-scale, gpsimd = collectives + rstd broadcast + epilogue slab sums.
  - LN stats via ones-vector matmuls on TensorE; mean subtraction rides fc1
    as an augmented K=1 row; rstd applied at PSUM eviction (broadcast via
    gpsimd.partition_broadcast); gamma folded into W1 host-side; output_b
    folded into the residual rows host-side. Matmuls bf16, fp32 PSUM.
"""

import numpy as np
import ml_dtypes

import concourse.bass as bass
import concourse.mybir as mybir
import concourse.tile as tile
from concourse import bacc
from concourse.bass_utils import run_bass_kernel_spmd

BF16 = mybir.dt.bfloat16
F32 = mybir.dt.float32
NPBF16 = ml_dtypes.bfloat16

H = 4096
T = 4096
I_FULL = 16384
NCORES = 8
I_S = I_FULL // NCORES   # 2048 intermediate cols per core
KC = H // 128            # 32 contraction chunks for fc1
IT = I_S // 128          # 16 i-tiles
HT = H // 512            # 8 output h-tiles
LN_EPS = 1e-5

TBS = [1024, 1024, 1024, 512, 512]          # token block sizes
BOFF = [sum(TBS[:i]) for i in range(len(TBS))]
OBS = [tb // NCORES for tb in TBS]          # owned rows per core per block
OOFF = [sum(OBS[:i]) for i in range(len(OBS))]
NBLK = len(TBS)
OWN_TOT = sum(OBS)                          # 512 owned rows per core
ECH = 256                                   # epilogue column chunk
NECH = H // ECH

_CACHE = {}


def _src_hash():
    import hashlib
    with open(__file__, "rb") as f:
        return int(hashlib.sha256(f.read()).hexdigest()[:8], 16)


def _vtag_shape(repeat, sim):
    return ((_src_hash() % 97) + 1, 2 * repeat + (1 if sim else 0) + 1)


def _build(repeat=1, sim=False):
    nc = bacc.Bacc("TRN2", target_bir_lowering=False, debug=False,
                   num_devices=NCORES)
    with tile.TileContext(nc) as tc:
        with tc.tile_pool(name="dram", bufs=1, space="DRAM") as dram:
            def ext_in(name, shape, dtype):
                return dram.tile(shape, dtype, kind="ExternalInput", name=name,
                                 uniquify=False)

            xt = ext_in("xt", [H, T], BF16)            # x^T
            rt = ext_in("rt", [H, T], BF16)            # residual^T
            w1t = ext_in("w1t", [IT, 128, KC, 128], BF16)   # gamma-folded W1 shard
            w2t = ext_in("w2t", [HT, 128, IT, 512], BF16)   # W2 shard
            gw1 = ext_in("gw1", [1, I_S], BF16)        # gamma @ W1 shard
            biasf = ext_in("biasf", [128, IT], F32)    # beta@W1 + b1, per i-tile cols
            xo = ext_in("xo", [OWN_TOT, H], F32)       # owned x rows (block-major)
            ro = ext_in("ro", [OWN_TOT, H], F32)       # owned residual rows + out_b
            vts = _vtag_shape(repeat, sim)
            vtag = ext_in("vtag", list(vts), F32)
            vscr = dram.tile(list(vts), F32, name="vscr")
            out = dram.tile([OWN_TOT, H], F32, kind="ExternalOutput",
                            name="out", uniquify=False)

            a2ain = [dram.tile([TBS[b], H], BF16, name=f"a2ain{b}")
                     for b in range(NBLK)]
            a2aout = [dram.tile([NCORES, OBS[b], H], BF16, name=f"a2aout{b}")
                      for b in range(NBLK)]

            from contextlib import ExitStack
            ctx = ExitStack()
            with ctx:
                pool = lambda name, bufs, **kw: ctx.enter_context(
                    tc.tile_pool(name=name, bufs=bufs, **kw))
                consts = pool("consts", 1)
                hpool = pool("hpool", 1)
                ipool = pool("ipool", 1)
                lnst = pool("lnst", 4)
                h2p = pool("h2p", 2)
                w1p = pool("w1p", 3)
                w2p = pool("w2p", 4)
                evp = pool("evp", 3)
                ev2p = pool("ev2p", 3)
                bcp = pool("bcp", 2)
                rows = pool("rows", 2)
                epip = pool("epip", 2)
                psst = pool("psst", 1, space="PSUM")
                mmps = pool("mmps", 2, space="PSUM")
                ones_col = consts.tile([128, 1], BF16)
                nc.vector.memset(ones_col[:], 1.0)
                eps_t = consts.tile([1, 1], F32)
                nc.vector.memset(eps_t[:], LN_EPS)
                gw1_sb = consts.tile([1, I_S], BF16)
                nc.sync.dma_start(out=gw1_sb[:], in_=gw1[:])
                biasf_sb = consts.tile([128, IT], F32)
                nc.sync.dma_start(out=biasf_sb[:], in_=biasf[:])
                nc.sync.dma_start(out=vscr[:], in_=vtag[:])

                # per-block live state (tiles shared between emission phases)
                st = {}

                def emit_lna(b):
                    """Activation loads + residual add + square for block b.
                    Emitted right after fc1(b-1): the hT WAR releases at
                    fc1(b-1)'s end, so these stream during fc2(b-1)."""
                    TB = TBS[b]
                    ts = slice(BOFF[b], BOFF[b] + TB)
                    hT = hpool.tile([128, KC, TB], BF16, name="hT")
                    h2s = []
                    for kc in range(KC):
                        ks = slice(kc * 128, (kc + 1) * 128)
                        xt_t = lnst.tile([128, TB], BF16, name="xt_t")
                        rt_t = lnst.tile([128, TB], BF16, name="rt_t")
                        nc.scalar.dma_start(out=xt_t[:], in_=xt[ks, ts])
                        nc.scalar.dma_start(out=rt_t[:], in_=rt[ks, ts])
                        nc.vector.tensor_add(hT[:, kc, :], xt_t[:], rt_t[:])
                        h2_t = h2p.tile([128, TB], BF16, name="h2_t")
                        nc.vector.tensor_mul(h2_t[:], hT[:, kc, :], hT[:, kc, :])
                        h2s.append(h2_t)
                    st[b] = {"hT": hT, "h2s": h2s}

                def make_stats_emitters(b):
                    """Per-kc stat-matmul emitters for block b, to be
                    interleaved into block b-1's fc2 PE stream."""
                    TB = TBS[b]
                    nsb = TB // 512
                    ps_s1 = [psst.tile([1, 512], F32, name=f"ps_s1_{s}")
                             for s in range(nsb)]
                    ps_s2 = [psst.tile([1, 512], F32, name=f"ps_s2_{s}")
                             for s in range(nsb)]
                    st[b]["ps_s1"] = ps_s1
                    st[b]["ps_s2"] = ps_s2
                    hT = st[b]["hT"]
                    h2s = st[b]["h2s"]

                    def emit_kc(kc):
                        for s in range(nsb):
                            ss = slice(s * 512, (s + 1) * 512)
                            nc.tensor.matmul(ps_s1[s][:], ones_col[:],
                                             hT[:, kc, ss],
                                             start=(kc == 0), stop=(kc == KC - 1))
                            nc.tensor.matmul(ps_s2[s][:], ones_col[:],
                                             h2s[kc][:, ss],
                                             start=(kc == 0), stop=(kc == KC - 1))
                    return [lambda kc=kc: emit_kc(kc) for kc in range(KC)]

                def emit_rows(b):
                    """LN statistics eviction: mu, rstd (+ broadcast)."""
                    nsb = TBS[b] // 512
                    negmu, rstd_bc = [], []
                    for s in range(nsb):
                        murow = rows.tile([1, 512], F32, name="murow")
                        nc.scalar.mul(out=murow[:], in_=st[b]["ps_s1"][s][:],
                                      mul=1.0 / H)
                        s2row = rows.tile([1, 512], F32, name="s2row")
                        nc.scalar.mul(out=s2row[:], in_=st[b]["ps_s2"][s][:],
                                      mul=1.0 / H)
                        nm = rows.tile([1, 512], BF16, name="negmu")
                        nc.scalar.mul(out=nm[:], in_=murow[:], mul=-1.0)
                        varrow = rows.tile([1, 512], F32, name="varrow")
                        nc.vector.tensor_mul(varrow[:], murow[:], murow[:])
                        nc.vector.tensor_sub(varrow[:], s2row[:], varrow[:])
                        # std = sqrt(var + eps), rstd = 1/std
                        nc.scalar.activation(out=varrow[:], in_=varrow[:],
                                             func=mybir.ActivationFunctionType.Sqrt,
                                             bias=eps_t[:])
                        nc.vector.reciprocal(out=varrow[:], in_=varrow[:])
                        bc = bcp.tile([128, 512], F32, name="rstd_bc")
                        nc.gpsimd.partition_broadcast(bc[:], varrow[:])
                        negmu.append(nm)
                        rstd_bc.append(bc)
                    st[b]["negmu"] = negmu
                    st[b]["rstd_bc"] = rstd_bc

                def emit_fc1(b):
                    TB = TBS[b]
                    nsb = TB // 512
                    hT = st[b]["hT"]
                    interT = ipool.tile([128, IT, TB], BF16, name="interT")
                    st[b]["interT"] = interT
                    for it in range(IT):
                        pss = [mmps.tile([128, 512], F32, name="psmm")
                               for _ in range(nsb)]
                        w1q = None
                        for kc in range(KC):
                            if kc % 8 == 0:
                                w1q = w1p.tile([128, 8, 128], BF16, name="w1q")
                                nc.sync.dma_start(
                                    out=w1q[:], in_=w1t[it, :, kc:kc + 8, :])
                            for s in range(nsb):
                                ss = slice(s * 512, (s + 1) * 512)
                                nc.tensor.matmul(pss[s][:], w1q[:, kc % 8, :],
                                                 hT[:, kc, ss],
                                                 start=(kc == 0), stop=False)
                        for s in range(nsb):
                            nc.tensor.matmul(pss[s][:],
                                             gw1_sb[:, it * 128:(it + 1) * 128],
                                             st[b]["negmu"][s][:],
                                             start=False, stop=True)
                            tmp = evp.tile([128, 512], BF16, name="tmp")
                            nc.vector.tensor_mul(tmp[:], pss[s][:],
                                                 st[b]["rstd_bc"][s][:])
                            nc.scalar.activation(
                                out=interT[:, it, s * 512:(s + 1) * 512],
                                in_=tmp[:],
                                func=mybir.ActivationFunctionType.Relu,
                                bias=biasf_sb[:, it:it + 1])

                def emit_fc2(b, interleave):
                    """fc2 + sprinkled emitters (next block's stat matmuls)."""
                    TB = TBS[b]
                    TT = TB // 128
                    interT = st[b]["interT"]
                    n_iter = HT * TT
                    n_int = len(interleave)
                    done = 0
                    it_i = 0
                    for ht in range(HT):
                        w2_th = [None, None]
                        for half in range(2):
                            w2_t = w2p.tile([128, IT // 2, 512], BF16,
                                            name="w2_t")
                            nc.sync.dma_start(
                                out=w2_t[:],
                                in_=w2t[ht, :, half * (IT // 2):(half + 1) * (IT // 2), :])
                            w2_th[half] = w2_t
                        for tt in range(TT):
                            ps2 = mmps.tile([128, 512], F32, name="psmm")
                            for ic in range(IT):
                                nc.tensor.matmul(
                                    ps2[:],
                                    interT[:, ic, tt * 128:(tt + 1) * 128],
                                    w2_th[ic // (IT // 2)][:, ic % (IT // 2), :],
                                    start=(ic == 0), stop=(ic == IT - 1))
                            ev2 = ev2p.tile([128, 512], BF16, name="ev2")
                            nc.scalar.copy(out=ev2[:], in_=ps2[:])
                            nc.sync.dma_start(
                                out=a2ain[b][tt * 128:(tt + 1) * 128,
                                             ht * 512:(ht + 1) * 512],
                                in_=ev2[:])
                            it_i += 1
                            want = n_int * it_i // n_iter
                            while done < want:
                                interleave[done]()
                                done += 1
                    while done < n_int:
                        interleave[done]()
                        done += 1
                    # hT/h2 of this block fully consumed; drop references
                    st[b].pop("h2s", None)

                def emit_cc(b):
                    if sim:
                        # timing-sim stand-in: token-sized copies (the real
                        # AllToAll runs on TOPSP silicon, not an engine ring;
                        # its latency is invisible to CoreSim either way)
                        for i in range(NCORES):
                            nc.sync.dma_start(
                                out=a2aout[b][i, 0:1, :],
                                in_=a2ain[b][i * OBS[b]:i * OBS[b] + 1, :])
                    else:
                        nc.gpsimd.collective_compute(
                            "AllToAll",
                            mybir.AluOpType.bypass,
                            replica_groups=[list(range(NCORES))],
                            ins=[a2ain[b].opt()],
                            outs=[a2aout[b].opt()],
                        )

                def emit_epi_loads(b):
                    ob = OBS[b]
                    o0 = OOFF[b]
                    tiles = []
                    for ch in range(NECH):
                        cs = slice(ch * ECH, (ch + 1) * ECH)
                        red = epip.tile([ob, NCORES, ECH], BF16, name="red")
                        nc.scalar.dma_start(
                            out=red[:],
                            in_=a2aout[b][:, :, cs].transpose([1, 0, 2]))
                        xo_t = epip.tile([ob, ECH], F32, name="xo_t")
                        nc.scalar.dma_start(out=xo_t[:], in_=xo[o0:o0 + ob, cs])
                        ro_t = epip.tile([ob, ECH], F32, name="ro_t")
                        nc.scalar.dma_start(out=ro_t[:], in_=ro[o0:o0 + ob, cs])
                        tiles.append((red, xo_t, ro_t))
                    st[b]["epi"] = tiles

                def emit_epi_adds(b, split=False):
                    accs = []
                    for ch, (red, xo_t, ro_t) in enumerate(st[b]["epi"]):
                        ob = red.shape[0]
                        # tail epilogue: nothing else runs, so split chunks
                        # across gpsimd and DVE to halve the critical path
                        eng = nc.vector if (split and ch % 2) else nc.gpsimd
                        acc = epip.tile([ob, ECH], F32, name="acc")
                        tmp2 = epip.tile([ob, ECH], F32, name="tmp2")
                        eng.tensor_add(acc[:], red[:, 0, :], red[:, 1, :])
                        eng.tensor_add(tmp2[:], red[:, 2, :], red[:, 3, :])
                        eng.tensor_add(acc[:], acc[:], tmp2[:])
                        eng.tensor_add(tmp2[:], red[:, 4, :], red[:, 5, :])
                        eng.tensor_add(acc[:], acc[:], tmp2[:])
                        eng.tensor_add(tmp2[:], red[:, 6, :], red[:, 7, :])
                        eng.tensor_add(acc[:], acc[:], tmp2[:])
                        eng.tensor_add(acc[:], acc[:], xo_t[:])
                        eng.tensor_add(acc[:], acc[:], ro_t[:])
                        accs.append(acc)
                    st[b]["accs"] = accs

                def emit_epi_outs(b):
                    ob = OBS[b]
                    o0 = OOFF[b]
                    for ch, acc in enumerate(st[b]["accs"]):
                        cs = slice(ch * ECH, (ch + 1) * ECH)
                        nc.scalar.dma_start(out=out[o0:o0 + ob, cs], in_=acc[:])
                    del st[b]

                for rep in range(repeat):
                    emit_lna(0)
                    stats_cbs = make_stats_emitters(0)
                    for cb in stats_cbs:
                        cb()
                    for b in range(NBLK):
                        emit_rows(b)
                        emit_fc1(b)
                        if b + 1 < NBLK:
                            emit_lna(b + 1)
                            next_stats = make_stats_emitters(b + 1)
                        else:
                            next_stats = []
                        if b >= 1:
                            emit_epi_loads(b - 1)
                            emit_epi_adds(b - 1)
                        emit_fc2(b, next_stats)
                        emit_cc(b)
                        if b >= 1:
                            emit_epi_outs(b - 1)
                    emit_epi_loads(NBLK - 1)
                    emit_epi_adds(NBLK - 1, split=True)
                    emit_epi_outs(NBLK - 1)
    nc.compile()
    return nc


def _own_idx(c):
    parts = []
    for b in range(NBLK):
        parts.append(BOFF[b] + c * OBS[b] + np.arange(OBS[b]))
    return np.concatenate(parts)


def _prep_inputs(x, residual, gamma, beta, inter_w, inter_b, output_w, output_b):
    f32 = np.float32
    x3 = np.ascontiguousarray(np.asarray(x, dtype=f32).reshape(T, H))
    r3 = np.ascontiguousarray(np.asarray(residual, dtype=f32).reshape(T, H))
    gamma = np.asarray(gamma, dtype=f32)
    beta = np.asarray(beta, dtype=f32)
    inter_w = np.asarray(inter_w, dtype=f32)
    inter_b = np.asarray(inter_b, dtype=f32)
    output_w = np.asarray(output_w, dtype=f32)
    output_b = np.asarray(output_b, dtype=f32)

    xt_np = np.ascontiguousarray(x3.T).astype(NPBF16)
    rt_np = np.ascontiguousarray(r3.T).astype(NPBF16)

    w1f = inter_w * gamma[:, None]
    gw1_full = gamma @ inter_w
    bias_full = beta @ inter_w + inter_b

    in_maps = []
    for c in range(NCORES):
        sl = slice(c * I_S, (c + 1) * I_S)
        w1s = w1f[:, sl]
        # [IT, 128(k in chunk), KC, 128(i)]
        w1tiles = np.ascontiguousarray(
            w1s.reshape(KC, 128, IT, 128).transpose(2, 1, 0, 3)).astype(NPBF16)
        w2s = output_w[sl, :]
        # [HT, 128(i in chunk), IT, 512(h)]
        w2tiles = np.ascontiguousarray(
            w2s.reshape(IT, 128, HT, 512).transpose(2, 1, 0, 3)).astype(NPBF16)
        gw1_c = gw1_full[sl].astype(NPBF16).reshape(1, I_S)
        biasf_c = np.ascontiguousarray(
            bias_full[sl].reshape(IT, 128).T).astype(f32)
        idx = _own_idx(c)
        in_maps.append({
            "vtag": np.zeros(_vtag_shape(1, False), dtype=f32),
            "xt": xt_np, "rt": rt_np,
            "w1t": w1tiles, "w2t": w2tiles,
            "gw1": gw1_c, "biasf": biasf_c,
            "xo": np.ascontiguousarray(x3[idx]),
            "ro": np.ascontiguousarray(r3[idx] + output_b[None, :]),
        })
    return in_maps


def get_nc(repeat=1, sim=False):
    key = ("nc", repeat, sim)
    if key not in _CACHE:
        _CACHE[key] = _build(repeat=repeat, sim=sim)
    return _CACHE[key]


def run(in_maps):
    nc = get_nc()
    return run_bass_kernel_spmd(nc, in_maps, core_ids=list(range(NCORES)))


def kernel(x, residual, gamma, beta, inter_w, inter_b, output_w, output_b):
    in_maps = _prep_inputs(x, residual, gamma, beta, inter_w, inter_b,
                           output_w, output_b)
    res = run(in_maps)
    out_full = np.empty((T, H), dtype=np.float32)
    for c in range(NCORES):
        out_full[_own_idx(c)] = res.results[c]["out"]
    return out_full.reshape(2, T // 2, H)
